# revision 13
# baseline (speedup 1.0000x reference)
"""CrossAttention kernel v6: host-fused kq = SCALE*k@Wq eliminates the whole
q path on device.

sim_h = (k_h Wq_h)^T x contracted over channels (fp8 DoubleRow, K=512), so
there is no q projection, no q psum drain (was 1/3 of DVE traffic), and no
k/v build on device (k@Wq and v are tiny host-side GEMMs, ~2% of FLOPs).
Attention runs all-fp8: es = exp(sim) in fp8 via one fused ACT op per
(head-pair, hh) with scale=1/KQ_SCALE folded in; av + denominators are
col-group-paired M=64 matmuls; softmax division via reciprocal+mul on DVE;
out-proj fp8 DoubleRow with x16 weight scaling and host-side /16.
"""

import numpy as np
import ml_dtypes

import concourse.bass as bass
import concourse.mybir as mybir
import concourse.tile as tile
from concourse import bacc
from concourse.bass_utils import run_bass_kernel_spmd

HEADS = 8
DIM_HEAD = 64
SCALE = DIM_HEAD ** -0.5
DIM = 512
N_CTX = 256
HW = 4096
CH = 512
NCHUNK = HW // CH  # 8
B = 8

F32 = mybir.dt.float32
BF16 = mybir.dt.bfloat16
F8 = mybir.dt.float8e4
WSCALE = 16.0    # host pre-scale on Wout so fp8e4m3 stays in normal range
KQ_SCALE = 32.0  # host pre-scale on kq (undone by exp's free scale)


def build_bass(loop_n=1):
    nc = bacc.Bacc(
        "TRN2",
        target_bir_lowering=False,
        debug=False,
        num_devices=B,
    )

    xb_d = nc.declare_dram_parameter("xb", [128, NCHUNK, 4, CH], F8, isOutput=False)
    xr_d = nc.declare_dram_parameter("xr", [128, NCHUNK, 4, CH], BF16, isOutput=False)
    kq_d = nc.declare_dram_parameter("kq8", [128, 4, HEADS, 2, 128], F8, isOutput=False)
    v_d = nc.declare_dram_parameter("v8", [128, 2, DIM], F8, isOutput=False)
    wo_d = nc.declare_dram_parameter("woutT", [128, 4, DIM], F8, isOutput=False)
    out_d = nc.declare_dram_parameter("out", [128, NCHUNK, 4, CH], BF16, isOutput=True)

    with tile.TileContext(nc) as tc:
        with (
            tc.tile_pool(name="wts", bufs=1) as wts,
            tc.tile_pool(name="xp", bufs=4) as xp,
            tc.tile_pool(name="rxp", bufs=4) as rxp,
            tc.tile_pool(name="ep", bufs=6) as ep,
            tc.tile_pool(name="rp", bufs=4) as rp,
            tc.tile_pool(name="ocp", bufs=3) as ocp,
            tc.tile_pool(name="outp", bufs=4) as outp,
            tc.tile_pool(name="psim", bufs=2, space="PSUM") as psim,
            tc.tile_pool(name="pav", bufs=2, space="PSUM") as pavp,
            tc.tile_pool(name="pS", bufs=1, space="PSUM") as pSp,
            tc.tile_pool(name="pq", bufs=1, space="PSUM") as pq,
        ):
            kq_sb = wts.tile([128, 4, HEADS, 2, 128], F8)
            # split per head so the first sim only waits for its own slice
            for h in range(HEADS):
                nc.sync.dma_start(out=kq_sb[:, :, h], in_=kq_d[:, :, h])
            v_sb = wts.tile([128, 2, DIM], F8)
            nc.sync.dma_start(out=v_sb, in_=v_d[:])
            wo_sb = wts.tile([128, 4, DIM], F8)
            nc.sync.dma_start(out=wo_sb, in_=wo_d[:])
            ones_sb = wts.tile([128, DIM_HEAD], F8)
            nc.vector.memset(ones_sb, 1.0)

            for _it in range(loop_n):

                def emit_attn_p(oc_t, xb_t, p):
                    # sim via kq (fp8 DR, K=512 over channels) + fused exp
                    es = []
                    for hh in range(2):
                        h = 2 * p + hh
                        pt = psim.tile([128, 2, CH], F32, tag="sim")
                        for j in range(2):
                            for g in range(2):
                                nc.tensor.matmul(
                                    pt[:, j, :],
                                    kq_sb[:, 2 * g:2 * g + 2, h, j, :],
                                    xb_t[:, 2 * g:2 * g + 2, :],
                                    start=(g == 0),
                                    stop=(g == 1),
                                    perf_mode=mybir.MatmulPerfMode.DoubleRow,
                                    skip_group_check=True,
                                )
                        e_sb = ep.tile([128, 2, CH], F8, tag="e")
                        nc.scalar.activation(
                            out=e_sb,
                            in_=pt,
                            func=mybir.ActivationFunctionType.Exp,
                            scale=1.0 / KQ_SCALE,
                        )
                        es.append(e_sb)
                    # av + denominators, col-tiled over hh pairs
                    av_t = pavp.tile([128, CH], F32, tag="pav")
                    pS_t = pSp.tile([128, CH], F32, tag="pS")
                    for kj in range(2):
                        for hh in range(2):
                            h = 2 * p + hh
                            h0 = hh * 64
                            nc.tensor.matmul(
                                av_t[h0:h0 + 64, :],
                                v_sb[:, kj, bass.ds(h * 64, 64)],
                                es[hh][:, kj, :],
                                start=(kj == 0),
                                stop=(kj == 1),
                                skip_group_check=True,
                            )
                    for kj in range(2):
                        for hh in range(2):
                            h0 = hh * 64
                            nc.tensor.matmul(
                                pS_t[h0:h0 + 64, :],
                                ones_sb,
                                es[hh][:, kj, :],
                                start=(kj == 0),
                                stop=(kj == 1),
                                skip_group_check=True,
                            )
                    r_sb = rp.tile([128, CH], F32, tag="r")
                    nc.vector.reciprocal_approx_fast(out=r_sb, in_=pS_t)
                    nc.vector.tensor_mul(out=oc_t[:, p, :], in0=av_t, in1=r_sb)

                def emit_out_m(o_sb, oc_sb, xr_t, m):
                    pt = pq.tile([128, CH], F32, tag="pq")
                    for g in range(2):
                        nc.tensor.matmul(
                            pt,
                            wo_sb[:, 2 * g:2 * g + 2, bass.ts(m, 128)],
                            oc_sb[:, 2 * g:2 * g + 2, :],
                            start=(g == 0),
                            stop=(g == 1),
                            perf_mode=mybir.MatmulPerfMode.DoubleRow,
                        )
                    nc.vector.tensor_add(
                        out=o_sb[:, m, :],
                        in0=pt,
                        in1=xr_t[:, m, :],
                    )

                # prologue: chunk 0 inputs
                xb_cur = xp.tile([128, 4, CH], F8, tag="xb")
                nc.sync.dma_start(out=xb_cur, in_=xb_d[:, 0])
                xr_cur = rxp.tile([128, 4, CH], BF16, tag="xr")
                nc.sync.dma_start(out=xr_cur, in_=xr_d[:, 0])

                # software pipeline: out-proj of chunk c-1 interleaves with
                # attention of chunk c, so ACT/PE/DVE stay co-resident.
                prev = None  # (oc_sb, xr, o_sb, chunk_idx)
                for c in range(NCHUNK):
                    if c + 1 < NCHUNK:
                        xb_nxt = xp.tile([128, 4, CH], F8, tag="xb")
                        nc.sync.dma_start(out=xb_nxt, in_=xb_d[:, c + 1])
                        xr_nxt = rxp.tile([128, 4, CH], BF16, tag="xr")
                        nc.sync.dma_start(out=xr_nxt, in_=xr_d[:, c + 1])
                    else:
                        xb_nxt = xr_nxt = None

                    oc_sb = ocp.tile([128, 4, CH], F8)
                    for p in range(4):
                        emit_attn_p(oc_sb, xb_cur, p)
                        if prev is not None:
                            emit_out_m(prev[2], prev[0], prev[1], p)
                    if prev is not None:
                        nc.gpsimd.dma_start(out=out_d[:, prev[3]], in_=prev[2])
                    o_sb = outp.tile([128, 4, CH], BF16)
                    prev = (oc_sb, xr_cur, o_sb, c)

                    xb_cur, xr_cur = xb_nxt, xr_nxt

                # epilogue: out-proj of the last chunk
                for m in range(4):
                    emit_out_m(prev[2], prev[0], prev[1], m)
                nc.gpsimd.dma_start(out=out_d[:, prev[3]], in_=prev[2])

    nc.compile()
    return nc


_NC_CACHE = None


def _get_nc():
    global _NC_CACHE
    if _NC_CACHE is None:
        _NC_CACHE = build_bass()
    return _NC_CACHE


def _shuffle_pcti(a_f32):
    return np.ascontiguousarray(
        a_f32.reshape(4, 128, NCHUNK, CH).transpose(1, 2, 0, 3)
    )


def make_in_maps(x, context, Wq, Wkv, Wout, bout):
    f = np.float32
    bf = ml_dtypes.bfloat16
    f8 = ml_dtypes.float8_e4m3

    def pm(wT, t, dt=bf):
        return np.ascontiguousarray(
            wT.reshape(t, 128, wT.shape[1]).transpose(1, 0, 2)
        ).astype(dt)

    Wq = np.asarray(Wq, dtype=f)
    Wkv = np.asarray(Wkv, dtype=f)
    Wout = np.asarray(Wout, dtype=f)
    woutT = pm(np.ascontiguousarray(Wout.T) * np.float32(WSCALE), 4, f8)
    bout = np.asarray(bout, dtype=f)
    in_maps = []
    for b in range(B):
        xf = np.ascontiguousarray(x[b].reshape(DIM, HW), dtype=f)
        k = context[b] @ Wkv[:DIM].T          # [256, 512]
        v = context[b] @ Wkv[DIM:].T          # [256, 512]
        # kq[c, h, ctx] = SCALE*KQ_SCALE * sum_dh Wq[64h+dh, c] k[ctx, 64h+dh]
        kq = np.einsum(
            "hdc,nhd->chn",
            Wq.reshape(HEADS, DIM_HEAD, DIM),
            k.reshape(N_CTX, HEADS, DIM_HEAD),
            optimize=True,
        ) * np.float32(SCALE * KQ_SCALE)      # [512c, 8h, 256ctx]
        kq8 = np.ascontiguousarray(
            kq.reshape(4, 128, HEADS, 2, 128).transpose(1, 0, 2, 3, 4)
        ).astype(f8)
        v8 = np.ascontiguousarray(
            v.reshape(2, 128, DIM).transpose(1, 0, 2)
        ).astype(f8)
        in_maps.append({
            "xb": _shuffle_pcti(xf).astype(f8),
            "xr": _shuffle_pcti((xf + bout[:, None]) * np.float32(WSCALE)).astype(bf),
            "kq8": kq8,
            "v8": v8,
            "woutT": woutT,
        })
    return in_maps


def postprocess(raw):
    return (
        raw.transpose(2, 0, 1, 3).reshape(DIM, 64, 64).astype(np.float32)
        * np.float32(1.0 / WSCALE)
    )


def kernel(x, context, Wq, Wkv, Wout, bout):
    x = np.asarray(x)
    context = np.asarray(context)
    nc = _get_nc()
    in_maps = make_in_maps(x, context, np.asarray(Wq), np.asarray(Wkv),
                           np.asarray(Wout), np.asarray(bout))
    res = run_bass_kernel_spmd(nc, in_maps, core_ids=list(range(B)))
    return np.stack([postprocess(res.results[b]["out"]) for b in range(B)], axis=0)


# revision 14
# speedup vs baseline: 1.0234x; 1.0234x over previous
"""CrossAttention kernel v6: host-fused kq = SCALE*k@Wq eliminates the whole
q path on device.

sim_h = (k_h Wq_h)^T x contracted over channels (fp8 DoubleRow, K=512), so
there is no q projection, no q psum drain (was 1/3 of DVE traffic), and no
k/v build on device (k@Wq and v are tiny host-side GEMMs, ~2% of FLOPs).
Attention runs all-fp8: es = exp(sim) in fp8 via one fused ACT op per
(head-pair, hh) with scale=1/KQ_SCALE folded in; av + denominators are
col-group-paired M=64 matmuls; softmax division via reciprocal+mul on DVE;
out-proj fp8 DoubleRow with x16 weight scaling and host-side /16.
"""

import numpy as np
import ml_dtypes

import concourse.bass as bass
import concourse.mybir as mybir
import concourse.tile as tile
from concourse import bacc
from concourse.bass_utils import run_bass_kernel_spmd

HEADS = 8
DIM_HEAD = 64
SCALE = DIM_HEAD ** -0.5
DIM = 512
N_CTX = 256
HW = 4096
CH = 512
NCHUNK = HW // CH  # 8
B = 8

F32 = mybir.dt.float32
BF16 = mybir.dt.bfloat16
F8 = mybir.dt.float8e4
WSCALE = 16.0    # host pre-scale on Wout so fp8e4m3 stays in normal range
KQ_SCALE = 32.0  # host pre-scale on kq (undone by exp's free scale)


def build_bass(loop_n=1):
    nc = bacc.Bacc(
        "TRN2",
        target_bir_lowering=False,
        debug=False,
        num_devices=B,
    )

    xb_d = nc.declare_dram_parameter("xb", [128, NCHUNK, 4, CH], F8, isOutput=False)
    xr_d = nc.declare_dram_parameter("xr", [128, NCHUNK, 4, CH], BF16, isOutput=False)
    kq_d = nc.declare_dram_parameter("kq8", [128, 4, HEADS, 2, 128], F8, isOutput=False)
    v_d = nc.declare_dram_parameter("v8", [128, 2, DIM], F8, isOutput=False)
    wo_d = nc.declare_dram_parameter("woutT", [128, 4, DIM], F8, isOutput=False)
    out_d = nc.declare_dram_parameter("out", [128, NCHUNK, 4, CH], BF16, isOutput=True)

    with tile.TileContext(nc) as tc:
        with (
            tc.tile_pool(name="wts", bufs=1) as wts,
            tc.tile_pool(name="xp", bufs=4) as xp,
            tc.tile_pool(name="rxp", bufs=4) as rxp,
            tc.tile_pool(name="ep", bufs=6) as ep,
            tc.tile_pool(name="rp", bufs=4) as rp,
            tc.tile_pool(name="ocp", bufs=3) as ocp,
            tc.tile_pool(name="outp", bufs=4) as outp,
            tc.tile_pool(name="psim", bufs=2, space="PSUM") as psim,
            tc.tile_pool(name="pav", bufs=2, space="PSUM") as pavp,
            tc.tile_pool(name="pS", bufs=1, space="PSUM") as pSp,
            tc.tile_pool(name="pq", bufs=1, space="PSUM") as pq,
        ):
            kq_sb = wts.tile([128, 4, HEADS, 2, 128], F8)
            nc.sync.dma_start(out=kq_sb, in_=kq_d[:])
            v_sb = wts.tile([128, 2, DIM], F8)
            nc.sync.dma_start(out=v_sb, in_=v_d[:])
            wo_sb = wts.tile([128, 4, DIM], F8)
            nc.sync.dma_start(out=wo_sb, in_=wo_d[:])
            ones_sb = wts.tile([128, DIM_HEAD], F8)
            nc.vector.memset(ones_sb, 1.0)

            for _it in range(loop_n):

                def emit_attn_p(oc_t, xb_t, p):
                    # sim via kq (fp8 DR, K=512 over channels) + fused exp
                    es = []
                    for hh in range(2):
                        h = 2 * p + hh
                        pt = psim.tile([128, 2, CH], F32, tag="sim")
                        for j in range(2):
                            for g in range(2):
                                nc.tensor.matmul(
                                    pt[:, j, :],
                                    kq_sb[:, 2 * g:2 * g + 2, h, j, :],
                                    xb_t[:, 2 * g:2 * g + 2, :],
                                    start=(g == 0),
                                    stop=(g == 1),
                                    perf_mode=mybir.MatmulPerfMode.DoubleRow,
                                    skip_group_check=True,
                                )
                        e_sb = ep.tile([128, 2, CH], F8, tag="e")
                        nc.scalar.activation(
                            out=e_sb,
                            in_=pt,
                            func=mybir.ActivationFunctionType.Exp,
                            scale=1.0 / KQ_SCALE,
                        )
                        es.append(e_sb)
                    # av + denominators, col-tiled over hh pairs
                    av_t = pavp.tile([128, CH], F32, tag="pav")
                    pS_t = pSp.tile([128, CH], F32, tag="pS")
                    for kj in range(2):
                        for hh in range(2):
                            h = 2 * p + hh
                            h0 = hh * 64
                            nc.tensor.matmul(
                                av_t[h0:h0 + 64, :],
                                v_sb[:, kj, bass.ds(h * 64, 64)],
                                es[hh][:, kj, :],
                                start=(kj == 0),
                                stop=(kj == 1),
                                skip_group_check=True,
                            )
                    for kj in range(2):
                        for hh in range(2):
                            h0 = hh * 64
                            nc.tensor.matmul(
                                pS_t[h0:h0 + 64, :],
                                ones_sb,
                                es[hh][:, kj, :],
                                start=(kj == 0),
                                stop=(kj == 1),
                                skip_group_check=True,
                            )
                    r_sb = rp.tile([128, CH], F32, tag="r")
                    nc.vector.reciprocal_approx_fast(out=r_sb, in_=pS_t)
                    nc.vector.tensor_mul(out=oc_t[:, p, :], in0=av_t, in1=r_sb)

                def emit_out_m(o_sb, oc_sb, xr_t, m):
                    pt = pq.tile([128, CH], F32, tag="pq")
                    for g in range(2):
                        nc.tensor.matmul(
                            pt,
                            wo_sb[:, 2 * g:2 * g + 2, bass.ts(m, 128)],
                            oc_sb[:, 2 * g:2 * g + 2, :],
                            start=(g == 0),
                            stop=(g == 1),
                            perf_mode=mybir.MatmulPerfMode.DoubleRow,
                        )
                    nc.vector.tensor_add(
                        out=o_sb[:, m, :],
                        in0=pt,
                        in1=xr_t[:, m, :],
                    )

                # prologue: chunk 0 inputs
                xb_cur = xp.tile([128, 4, CH], F8, tag="xb")
                nc.sync.dma_start(out=xb_cur, in_=xb_d[:, 0])
                xr_cur = rxp.tile([128, 4, CH], BF16, tag="xr")
                nc.sync.dma_start(out=xr_cur, in_=xr_d[:, 0])

                # software pipeline: out-proj of chunk c-1 interleaves with
                # attention of chunk c, so ACT/PE/DVE stay co-resident.
                prev = None  # (oc_sb, xr, o_sb, chunk_idx)
                for c in range(NCHUNK):
                    if c + 1 < NCHUNK:
                        xb_nxt = xp.tile([128, 4, CH], F8, tag="xb")
                        nc.sync.dma_start(out=xb_nxt, in_=xb_d[:, c + 1])
                        xr_nxt = rxp.tile([128, 4, CH], BF16, tag="xr")
                        nc.sync.dma_start(out=xr_nxt, in_=xr_d[:, c + 1])
                    else:
                        xb_nxt = xr_nxt = None

                    oc_sb = ocp.tile([128, 4, CH], F8)
                    for p in range(4):
                        emit_attn_p(oc_sb, xb_cur, p)
                        if prev is not None:
                            emit_out_m(prev[2], prev[0], prev[1], p)
                    if prev is not None:
                        nc.gpsimd.dma_start(out=out_d[:, prev[3]], in_=prev[2])
                    o_sb = outp.tile([128, 4, CH], BF16)
                    prev = (oc_sb, xr_cur, o_sb, c)

                    xb_cur, xr_cur = xb_nxt, xr_nxt

                # epilogue: out-proj of the last chunk
                for m in range(4):
                    emit_out_m(prev[2], prev[0], prev[1], m)
                nc.gpsimd.dma_start(out=out_d[:, prev[3]], in_=prev[2])

    nc.compile()
    return nc


_NC_CACHE = None


def _get_nc():
    global _NC_CACHE
    if _NC_CACHE is None:
        _NC_CACHE = build_bass()
    return _NC_CACHE


def _shuffle_pcti(a_f32):
    return np.ascontiguousarray(
        a_f32.reshape(4, 128, NCHUNK, CH).transpose(1, 2, 0, 3)
    )


def make_in_maps(x, context, Wq, Wkv, Wout, bout):
    f = np.float32
    bf = ml_dtypes.bfloat16
    f8 = ml_dtypes.float8_e4m3

    def pm(wT, t, dt=bf):
        return np.ascontiguousarray(
            wT.reshape(t, 128, wT.shape[1]).transpose(1, 0, 2)
        ).astype(dt)

    Wq = np.asarray(Wq, dtype=f)
    Wkv = np.asarray(Wkv, dtype=f)
    Wout = np.asarray(Wout, dtype=f)
    woutT = pm(np.ascontiguousarray(Wout.T) * np.float32(WSCALE), 4, f8)
    bout = np.asarray(bout, dtype=f)
    in_maps = []
    for b in range(B):
        xf = np.ascontiguousarray(x[b].reshape(DIM, HW), dtype=f)
        k = context[b] @ Wkv[:DIM].T          # [256, 512]
        v = context[b] @ Wkv[DIM:].T          # [256, 512]
        # kq[c, h, ctx] = SCALE*KQ_SCALE * sum_dh Wq[64h+dh, c] k[ctx, 64h+dh]
        kq = np.einsum(
            "hdc,nhd->chn",
            Wq.reshape(HEADS, DIM_HEAD, DIM),
            k.reshape(N_CTX, HEADS, DIM_HEAD),
            optimize=True,
        ) * np.float32(SCALE * KQ_SCALE)      # [512c, 8h, 256ctx]
        kq8 = np.ascontiguousarray(
            kq.reshape(4, 128, HEADS, 2, 128).transpose(1, 0, 2, 3, 4)
        ).astype(f8)
        v8 = np.ascontiguousarray(
            v.reshape(2, 128, DIM).transpose(1, 0, 2)
        ).astype(f8)
        in_maps.append({
            "xb": _shuffle_pcti(xf).astype(f8),
            "xr": _shuffle_pcti((xf + bout[:, None]) * np.float32(WSCALE)).astype(bf),
            "kq8": kq8,
            "v8": v8,
            "woutT": woutT,
        })
    return in_maps


def postprocess(raw):
    return (
        raw.transpose(2, 0, 1, 3).reshape(DIM, 64, 64).astype(np.float32)
        * np.float32(1.0 / WSCALE)
    )


def kernel(x, context, Wq, Wkv, Wout, bout):
    x = np.asarray(x)
    context = np.asarray(context)
    nc = _get_nc()
    in_maps = make_in_maps(x, context, np.asarray(Wq), np.asarray(Wkv),
                           np.asarray(Wout), np.asarray(bout))
    res = run_bass_kernel_spmd(nc, in_maps, core_ids=list(range(B)))
    return np.stack([postprocess(res.results[b]["out"]) for b in range(B)], axis=0)
